# revision 58
# baseline (speedup 1.0000x reference)
"""Single-head attention block (Q/K/V/O projections + softmax attention) on
8 Trainium2 NeuronCores.

Problem: x [16, 2048, 512] fp32; four 512x512 projections (torch convention
y = x @ W.T + b); scores = Q @ K.T / sqrt(512); softmax over keys;
out = attn @ V; y = out @ Wo.T + bo.

Sharding: pure data-parallel over batch — each of the 8 cores computes 2 of
the 16 batches end-to-end. No collectives.

Algebraic restructuring (softmax is invariant to adding any function of the
query row, so those terms are dropped):
  scores = (x Wq^T + bq)(x Wk^T + bk)^T / sqrt(D)
         ~ x A x^T + w[k]      with A = Wq^T Wk / sqrt(D)  (precomputed once)
                                    w = x (Wk^T bq) / sqrt(D)
  out = attn (x Wv^T + bv);  y = out Wo^T + bo
      = attn x B + c          with B = Wv^T Wo^T (once), c = bv Wo^T + bo
This removes the Q, K and V projections entirely: per batch only
  HT[d',q] = A-tiles.T @ xT  + v[d']   (v = Wk^T bq / sqrt(D) folded in as
                                        the ACT bias at HT eviction, which
                                        absorbs w into the scores directly)
  scoresT[k,q] = xT-tiles.T @ HT       -> exp(psum * 1/SA) on ACT
  ZT[d,q] += x-tiles.T @ attnT ;  rowsum via DVE adds + ones-matmuls
  y[q,g] = (ZT-tiles.T @ B) * (1/rs) + c

The HT and scores matmuls run in fp8e4 with MatmulPerfMode.DoubleRow (both
operands hold d-tile pairs as dim-1 of a 3D AP, contracting 256 rows per
512-cycle stream - 2x the fp32r rate); A/HT are pre-scaled by SA=1024 into
e4m3's normal range and the exp applies 1/SA. Z and the y projection run in
bf16. End-to-end rel err ~1.35e-2 vs the 2e-2 tolerance. Scores run two
k-tiles ahead of their exp; DoubleRow weight loads interleave between Z
streams; HT(qc+1), the previous chunk's epilogue (one q-tile per kt), x
DMA/transpose prefetch and next-batch staging occupy fixed kt slots. A
matmul burst at kernel start flips the PE HAM clock-gate to 2.4 GHz while
the first DMAs are in flight.
"""

import os
from contextlib import ExitStack

import numpy as np

import concourse.bass as bass
import concourse.tile as tile
from concourse import bacc, mybir
from concourse.bass_utils import run_bass_kernel_spmd
from concourse.masks import make_identity

N_CORES = 8
B, S, D = 16, 2048, 512
BPC = B // N_CORES  # batches per core
P = 128
ND = D // P         # 4   tiles over d/e/f dims
NS = S // P         # 16  tiles over s (= q = k) dim
QC = 512            # s/q-chunk width (PSUM bank)
NQC = S // QC       # 4
TPC = QC // P       # 4   128-tiles per chunk
NPAIR = ND // 2     # 2   d-tile pairs for DoubleRow
SCALE = float(1.0 / np.sqrt(D))
SA = 1024.0         # fp8 pre-scale for A / HT (keeps e4m3 in normal range)

F32 = mybir.dt.float32
F32R = mybir.dt.float32r
BF16 = mybir.dt.bfloat16
FP8 = mybir.dt.float8e4
DR = mybir.MatmulPerfMode.DoubleRow
AFT = mybir.ActivationFunctionType
ALU = mybir.AluOpType


def _emit(tc, x_ap, w_aps, b_aps, y_ap, fast_mm=True):
    nc = tc.nc
    MDT = F32R  # dtype of fp32-path matmul-feeding SBUF tiles
    ctx = ExitStack()
    with ctx:
        # ---- pools ----
        consts = ctx.enter_context(tc.tile_pool(name="consts", bufs=1))
        stage = ctx.enter_context(tc.tile_pool(name="stage", bufs=4))
        xs_pool = ctx.enter_context(tc.tile_pool(name="xs", bufs=12))
        ab_pool = ctx.enter_context(tc.tile_pool(name="ab", bufs=1))
        xt_pool = ctx.enter_context(tc.tile_pool(name="xt", bufs=2))
        xn_pool = ctx.enter_context(tc.tile_pool(name="xn", bufs=NS + 8))
        ht_pool = ctx.enter_context(tc.tile_pool(name="ht", bufs=2 * NPAIR))
        oc_pool = ctx.enter_context(tc.tile_pool(name="oc", bufs=12))
        at_pool = ctx.enter_context(tc.tile_pool(name="at", bufs=7))
        acc_pool = ctx.enter_context(tc.tile_pool(name="acc", bufs=4))
        y_pool = ctx.enter_context(tc.tile_pool(name="y", bufs=3))
        rs_pool = ctx.enter_context(tc.tile_pool(name="rs", bufs=2))
        ppt = ctx.enter_context(tc.tile_pool(name="ppt", bufs=4, space="PSUM"))
        ppo = ctx.enter_context(tc.tile_pool(name="ppo", bufs=4, space="PSUM"))

        def pt_tile():
            return ppt.tile([P, QC], F32, tag="ppt", name="pt")

        # ---- constants ----
        ones_bf = consts.tile([P, P], mybir.dt.bfloat16, tag="ones_bf")
        nc.vector.memset(ones_bf[:], 1.0)

        def filler(n=1):
            # bf16 no-op matmuls that keep the PE HAM activity window busy
            # through DMA-bound stretches so the clock gate stays at 2.4 GHz
            for _ in range(n):
                ps = pt_tile()
                nc.tensor.matmul(
                    ps[:, 0:P], ones_bf[:], ones_bf[:], start=True, stop=True
                )

        def ldw_filler(n=1):
            # weight-load-only PE activity: no PSUM slot, no output, just keeps
            # the HAM window busy while DMAs land (b0 head is DMA-bound)
            for _ in range(n):
                nc.tensor.ldweights(ones_bf[:])

        # Dense matmul burst: ~4.5us of sustained PE activity flips the PE HAM
        # clock-gate to 8/8 (2.4 GHz) while the first DMAs are in flight.
        filler(20)
        ident = consts.tile([P, P], F32, tag="ident")
        make_identity(nc, ident[:])
        ident_r = consts.tile([P, P], MDT, tag="ident_r")
        nc.vector.tensor_copy(ident_r[:], ident[:])
        ones_stage = stage.tile([P, P], F32, tag="stage", name="ones_stage")
        nc.vector.memset(ones_stage[:], 1.0)
        ones_col = consts.tile([P, 1], MDT, tag="ones_col")
        nc.vector.tensor_copy(ones_col[:], ones_stage[:, 0:1])
        ones_row = consts.tile([1, P], MDT, tag="ones_row")
        nc.vector.tensor_copy(ones_row[:], ones_stage[0:1, :])

        def row_to_col(row_ap, dst_ap, scale=None):
            """[1, 128] SBUF row -> [128, 1] SBUF column via PE transpose."""
            ps = pt_tile()
            nc.tensor.transpose(ps[:, 0:1], row_ap.bitcast(F32), ident[0:1, 0:1])
            if scale is None:
                nc.vector.tensor_copy(dst_ap, ps[:, 0:1])
            else:
                nc.vector.tensor_scalar_mul(dst_ap, ps[:, 0:1], scale)

        def load_bias_row(nm):
            st = stage.tile([1, D], F32, tag="stage", name="brow")
            nc.sync.dma_start(st[:], b_aps[nm][None, :])
            return st

        def load_wnat(nm):
            """Weight, natural [row, col] layout, DMA'd straight into f32r."""
            tiles = []
            for rt in range(ND):
                t = oc_pool.tile([P, D], MDT, tag="oc", name=f"{nm}n{rt}")
                nc.sync.dma_start(
                    t[:], w_aps[nm][P * rt : P * (rt + 1), :].bitcast(F32R)
                )
                tiles.append(t)
            return tiles

        def load_wqwk():
            """Wq/Wk interleaved per row-tile so the et-pipelined A setup can
            start its first accumulation ~1.5us after the first DMAs land."""
            wq, wk = [], []
            for rt in range(ND):
                for nm, lst in (("Wq", wq), ("Wk", wk)):
                    t = oc_pool.tile([P, D], MDT, tag="oc", name=f"{nm}i{rt}")
                    nc.sync.dma_start(
                        t[:], w_aps[nm][P * rt : P * (rt + 1), :].bitcast(F32R)
                    )
                    lst.append(t)
            return wq, wk

        # ---- one-time weight setup ----
        # A8[i][p, two*D + dp] = SA*SCALE*(Wq^T Wk)[128*(2i+two)+p, dp]
        A8 = [
            ab_pool.tile([P, 2 * D], FP8, tag=f"A{j}", name=f"A{j}")
            for j in range(NPAIR)
        ]
        Bm = [
            ab_pool.tile([P, D], BF16, tag=f"B{j}", name=f"B{j}")
            for j in range(ND)
        ]
        # v_sa[:, t] = SA * SCALE * (bq^T Wk)[128t : 128(t+1)]  (ACT bias col)
        v_sa = consts.tile([P, ND], F32, tag="v_sa")
        w_setup = {}

        def a8_view(i, dpt):
            return A8[i][:].rearrange("p (two dp) -> p two dp", two=2)[
                :, :, P * dpt : P * (dpt + 1)
            ]

        def setup_part1(wq, wk):
            # A = Wq^T Wk * SCALE * SA -> fp8 pairs ;  v = (Wk^T bq) * SCALE*SA
            # et-outer across 4 PSUM banks: each accumulation step waits only
            # on its own Wq/Wk row-tile pair, pipelining against the DMAs.
            bq_row = load_bias_row("bq")
            ps_a = [pt_tile() for _ in range(ND)]
            for et in range(ND):
                for dt_ in range(ND):
                    nc.tensor.matmul(
                        ps_a[dt_][:],
                        wq[et][:, P * dt_ : P * (dt_ + 1)],
                        wk[et][:],
                        start=(et == 0),
                        stop=(et == ND - 1),
                    )
            for dt_ in range(ND):
                nc.vector.tensor_scalar_mul(
                    A8[dt_ // 2][:, (dt_ % 2) * D : (dt_ % 2 + 1) * D],
                    ps_a[dt_][:],
                    SCALE * SA,
                )
            bq_col = consts.tile([P, ND], MDT, tag="bq_col")
            for t in range(ND):
                row_to_col(bq_row[0:1, P * t : P * (t + 1)], bq_col[:, t : t + 1])
            psv = pt_tile()
            for et in range(ND):
                nc.tensor.matmul(
                    psv[0:1, :],
                    bq_col[:, et : et + 1],
                    wk[et][:],
                    start=(et == 0),
                    stop=(et == ND - 1),
                )
            v_row = stage.tile([1, D], F32, tag="stage", name="v_row")
            nc.vector.tensor_scalar_mul(v_row[:], psv[0:1, :], SCALE * SA)
            for t in range(ND):
                row_to_col(v_row[0:1, P * t : P * (t + 1)], v_sa[:, t : t + 1])

        def setup_part2(wv, wo):
            # B = Wv^T Wo^T ;  c = bv Wo^T + bo  (broadcast to 128 rows)
            woT = [
                oc_pool.tile([P, D], MDT, tag="oc", name=f"WoT{j}")
                for j in range(ND)
            ]
            for gt in range(ND):
                for ft in range(ND):
                    ps = pt_tile()
                    nc.tensor.transpose(
                        ps[:, 0:P],
                        wo[gt][:, P * ft : P * (ft + 1)].bitcast(F32),
                        ident[:],
                    )
                    nc.vector.tensor_copy(woT[ft][:, P * gt : P * (gt + 1)], ps[:, 0:P])
            for dt_ in range(ND):
                ps = pt_tile()
                for ft in range(ND):
                    nc.tensor.matmul(
                        ps[:],
                        wv[ft][:, P * dt_ : P * (dt_ + 1)],
                        woT[ft][:],
                        start=(ft == 0),
                        stop=(ft == ND - 1),
                    )
                nc.vector.tensor_copy(Bm[dt_][:], ps[:])
            bv_row = load_bias_row("bv")
            bo_row = load_bias_row("bo")
            bv_col = stage.tile([P, ND], MDT, tag="stage", name="bv_col")
            for t in range(ND):
                row_to_col(bv_row[0:1, P * t : P * (t + 1)], bv_col[:, t : t + 1])
            psc = pt_tile()
            for ft in range(ND):
                nc.tensor.matmul(
                    psc[0:1, :],
                    bv_col[:, ft : ft + 1],
                    woT[ft][:],
                    start=(ft == 0),
                    stop=(ft == ND - 1),
                )
            c_row = stage.tile([1, D], MDT, tag="stage", name="c_row")
            nc.vector.tensor_add(c_row[:], psc[0:1, :], bo_row[0:1, :])
            psb = pt_tile()
            nc.tensor.matmul(psb[:], ones_row[:], c_row[:], start=True, stop=True)
            c_bc = consts.tile([P, D], F32, tag="c_bc")
            nc.vector.tensor_copy(c_bc[:], psb[:])
            w_setup["c_bc"] = c_bc

        # per-q-chunk epilogue. The PSUM-freeing evictions (ZT chunk -> SBUF,
        # rowsum -> SBUF) are emitted immediately at chunk end; the PE-side tail
        # (1/rs transposes + y projection) is deferred into the next chunk's
        # kt-loop so the PE never drains between chunks.
        state = {"pending": None, "head": None}

        def evict_chunk(b, qc, po, pr):
            # po evictions first (they free the PSUM banks the next chunk's
            # first Z matmuls write), split DVE/ACT so neither queue serializes
            oc = [
                oc_pool.tile([P, QC], BF16, tag="oc", name="oc") for _ in range(ND)
            ]
            for dt_ in range(ND):
                if dt_ % 2:
                    nc.scalar.activation(oc[dt_][:], po[dt_][:], AFT.Copy)
                else:
                    nc.vector.tensor_copy(oc[dt_][:], po[dt_][:])
            rsrow = rs_pool.tile([1, QC], F32, tag="rs", name="rsrow")
            nc.vector.tensor_copy(rsrow[:], pr[:])
            return (b, qc, oc, rsrow)

        def emit_epilogue_head(b, qc, oc, rsrow):
            rsT = rs_pool.tile([P, TPC], F32, tag="rsT", name="rsT")
            for j in range(TPC):
                row_to_col(rsrow[0:1, P * j : P * (j + 1)], rsT[:, j : j + 1])
            rsr = rs_pool.tile([P, TPC], F32, tag="rsr", name="rsr")
            nc.vector.reciprocal(rsr[:], rsT[:])
            return (b, qc, oc, rsr)

        def emit_epilogue_group(j, b, qc, oc, rsr):
            i = TPC * qc + j
            ps = pt_tile()
            for dt_ in range(ND):
                nc.tensor.matmul(
                    ps[:],
                    oc[dt_][:, P * j : P * (j + 1)],
                    Bm[dt_][:],
                    start=(dt_ == 0),
                    stop=(dt_ == ND - 1),
                )
            ysb = y_pool.tile([P, D], F32, tag="y", name="ysb")
            nc.vector.scalar_tensor_tensor(
                ysb[:],
                ps[:],
                rsr[:, j : j + 1],
                w_setup["c_bc"][:],
                op0=ALU.mult,
                op1=ALU.add,
            )
            nc.sync.dma_start(y_ap[b, P * i : P * (i + 1), :], ysb[:])

        def emit_epilogue(b, qc, oc, rsrow):
            args = emit_epilogue_head(b, qc, oc, rsrow)
            for j in range(TPC):
                emit_epilogue_group(j, *args)

        # ---- per batch ----
        # xT is one flat [128, ND*S] fp8 tile per batch, d-tile-major: column
        # block dt*S + s holds x[s, dt*128+p]. One strided DVE copy evicts a
        # whole x-tile's 4 transposed blocks at once (f32r psum -> fp8 sbuf).
        xTs = [
            xt_pool.tile([P, ND * S], FP8, tag="xt", name=f"xT{b}")
            for b in range(BPC)
        ]
        xNs = [
            [xn_pool.tile([P, D], BF16, tag="xn", name=f"xN{b}") for _ in range(NS)]
            for b in range(BPC)
        ]
        dma_done = [set() for _ in range(BPC)]
        tp_done = [set() for _ in range(BPC)]
        xst_tiles = {}

        def xt_pair(bb, i, lo, hi):
            # [128, 2, hi-lo] fp8 view of d-tile pair i (pair stride = S)
            return xTs[bb][:].rearrange("p (dt s) -> p dt s", dt=ND)[
                :, 2 * i : 2 * i + 2, lo:hi
            ]

        def emit_x_dma(bb, sc):
            # DMA one 512-wide s-chunk of batch bb into f32r staging tiles
            if sc in dma_done[bb]:
                return
            dma_done[bb].add(sc)
            for j in range(TPC):
                i = TPC * sc + j
                xst = xs_pool.tile([P, D], MDT, tag="xs", name="xst")
                nc.sync.dma_start(
                    xst[:], x_ap[bb, P * i : P * (i + 1), :].bitcast(F32R)
                )
                xst_tiles[(bb, i)] = xst

        def emit_x_tp(bb, i):
            # fork one staged x-tile: PE-transpose -> fp8 xT (DVE evict),
            # ACT-convert -> bf16 xN
            if i in tp_done[bb]:
                return
            tp_done[bb].add(i)
            xst = xst_tiles.pop((bb, i))
            ps = ppt.tile([P, QC], MDT, tag="ppt", name="ptr")
            for dt_ in range(ND):
                nc.tensor.transpose(
                    ps[:, P * dt_ : P * (dt_ + 1)],
                    xst[:, P * dt_ : P * (dt_ + 1)],
                    ident_r[:],
                )
            nc.vector.tensor_copy(
                xTs[bb][:].rearrange("p (dt s) -> p dt s", dt=ND)[
                    :, :, P * i : P * (i + 1)
                ],
                ps[:].rearrange("p (dt c) -> p dt c", dt=ND),
            )
            nc.scalar.activation(xNs[bb][i][:], xst[:], AFT.Copy)

        def emit_x_tp_half(bb, sc, half):
            for j in (0, 1):
                emit_x_tp(bb, TPC * sc + 2 * half + j)

        HTs = [[None] * NQC for _ in range(BPC)]

        def emit_ht_dpt(bb, hsc, dpt):
            # One 128-row slice of HT for q-chunk hsc of batch bb: 2
            # DoubleRow MMs (contract d-tile pairs of SA*A against xT
            # pairs) + an ACT eviction that adds SA*v[d'] (folds w into
            # the scores).
            if dpt == 0:
                HTs[bb][hsc] = [
                    ht_pool.tile([P, 2 * QC], FP8, tag="ht", name="HT")
                    for _ in range(NPAIR)
                ]
            ps = pt_tile()
            for i in range(NPAIR):
                nc.tensor.matmul(
                    ps[:],
                    a8_view(i, dpt),
                    xt_pair(bb, i, QC * hsc, QC * (hsc + 1)),
                    start=(i == 0),
                    stop=(i == NPAIR - 1),
                    perf_mode=DR,
                )
            nc.scalar.activation(
                HTs[bb][hsc][dpt // 2][:, (dpt % 2) * QC : (dpt % 2 + 1) * QC],
                ps[:],
                AFT.Identity,
                bias=v_sa[:, dpt : dpt + 1],
            )

        for b in range(BPC):
            xN = xNs[b]
            if b == 0:
                # Head: Wq/Wk DMAs first (A = Wq^T Wk heads the longest
                # dependency chain A -> HT(0) -> scores); x chunks 0-2 ride
                # behind them; chunk-0 transposes and the et-pipelined A
                # matmuls interleave against the landing DMAs.
                wsetup = getattr(_emit, "_ws", {})
                _emit._ws = wsetup
                wq, wk = load_wqwk()
                emit_x_dma(0, 0)
                emit_x_dma(0, 1)
                setup_part1(wq, wk)
                wsetup["wv"] = load_wnat("Wv")
                wsetup["wo"] = load_wnat("Wo")
                for i in range(TPC):
                    emit_x_tp(0, i)
            if HTs[b][0] is None:
                for dpt in range(ND):
                    emit_ht_dpt(b, 0, dpt)
            for qc in range(NQC):
                po = [
                    ppo.tile([P, QC], F32, tag="ppo", name="po") for _ in range(ND)
                ]
                # software-pipelined: scoresT(kt+1) overlaps exp(kt) on ACT
                pss = [None] * NS
                at = [None] * NS
                acc = [None, None]

                def ht_view(i):
                    return HTs[b][qc][i][:].rearrange(
                        "p (two q) -> p two q", two=2
                    )

                def scores_mm(kt, i):
                    # one DoubleRow scores MM; i==0 allocates the PSUM tile
                    if i == 0:
                        pss[kt] = pt_tile()
                    nc.tensor.matmul(
                        pss[kt][:],
                        xt_pair(b, i, P * kt, P * (kt + 1)),
                        ht_view(i),
                        start=(i == 0),
                        stop=(i == NPAIR - 1),
                        perf_mode=DR,
                    )

                scores_mm(0, 0)
                scores_mm(0, 1)
                scores_mm(1, 0)
                scores_mm(1, 1)
                for kt in range(NS):
                    a = at_pool.tile([P, QC], BF16, tag="at", name="at")
                    nc.scalar.activation(
                        a[:], pss[kt][:], AFT.Exp, scale=1.0 / SA
                    )
                    at[kt] = a
                    nxt = kt + 2 < NS

                    def z_mm(dt_):
                        nc.tensor.matmul(
                            po[dt_][:],
                            xN[kt][:, P * dt_ : P * (dt_ + 1)],
                            at[kt][:],
                            start=(kt == 0),
                            stop=(kt == NS - 1),
                        )

                    # PE stream interleaves the LDW-heavy DoubleRow MMs (256
                    # weight cols each) between the Z MMs; scores run TWO
                    # k-tiles ahead so exp(kt+1) has a full iteration of ACT
                    # slack before its Z matmuls need the result.
                    if nxt:
                        scores_mm(kt + 2, 0)
                    z_mm(0)
                    z_mm(1)
                    if nxt:
                        scores_mm(kt + 2, 1)
                    z_mm(2)
                    # HT(qc+1) slices land at kts 2,3,12,13 (between Z MMs so
                    # their 256-col weight loads hide under the streams)
                    if qc + 1 < NQC:
                        if kt in (2, 3):
                            emit_ht_dpt(b, qc + 1, kt - 2)
                        elif kt in (12, 13):
                            emit_ht_dpt(b, qc + 1, kt - 10)
                    elif b + 1 < BPC and kt >= 12:
                        emit_ht_dpt(b + 1, 0, kt - 12)
                    z_mm(3)
                    # the previous q-chunk's deferred epilogue: 1/rs at kt 2,
                    # one y-projection quarter at each of kts 5,7,9,11 -- so
                    # the PE never drains between chunks and no single kt
                    # congests. The final flush happens after the b loop.
                    if state["pending"] is not None:
                        if kt == 2:
                            state["head"] = emit_epilogue_head(*state["pending"])
                        elif kt in (5, 7, 9, 11):
                            emit_epilogue_group((kt - 5) // 2, *state["head"])
                            if kt == 11:
                                state["pending"] = None
                                state["head"] = None
                    # x prefetch: DMA-dependent transposes go at the END of
                    # the kt body so already-queued MMs run while DMA lands.
                    # x-tiles are transposed >=1 kt before the two-ahead
                    # scores matmuls read them; the last chunk stages the
                    # next batch's chunks 0/1.
                    if qc == 0:
                        if kt in (0, 1):
                            if kt == 0:
                                emit_x_dma(b, 2)
                            emit_x_tp_half(b, 1, kt)
                        elif kt == 2:
                            emit_x_dma(b, 3)
                        elif kt in (4, 5):
                            emit_x_tp_half(b, 2, kt - 4)
                        elif kt in (8, 9):
                            emit_x_tp_half(b, 3, kt - 8)
                    if qc == NQC - 1 and b + 1 < BPC:
                        if kt == 1:
                            emit_x_dma(b + 1, 0)
                        elif kt == 5:
                            emit_x_dma(b + 1, 1)
                        elif kt in (6, 7):
                            emit_x_tp_half(b + 1, 0, kt - 6)
                        elif kt in (10, 11):
                            emit_x_tp_half(b + 1, 1, kt - 10)
                            if kt == 11:
                                emit_x_dma(b + 1, 2)
                    # rowsum over k runs on DVE with two interleaved
                    # accumulators; at[14]/at[15] skip the DVE chain and ride
                    # the ones-matmul group directly so the chunk's final PE
                    # work never waits on the vector engine.
                    if kt in (2, 3):
                        j = kt - 2
                        acc[j] = acc_pool.tile([P, QC], MDT, tag="acc", name="acc")
                        nc.vector.tensor_add(acc[j][:], at[j][:], at[kt][:])
                    elif 4 <= kt < NS - 2:
                        j = kt % 2
                        nc.vector.tensor_add(acc[j][:], acc[j][:], at[kt][:])
                    # B / c are first needed by qc0's epilogue (flushed at
                    # qc1 kt==2): compute them inside qc0's dense kt-loop
                    if b == 0 and qc == 0 and kt == 11:
                        wsetup = _emit._ws
                        setup_part2(wsetup.pop("wv"), wsetup.pop("wo"))
                pr = ppt.tile([1, QC], F32, tag="ppt", name="pr")
                for j in range(2):
                    nc.tensor.matmul(
                        pr[:],
                        ones_col[:],
                        acc[j][:],
                        start=(j == 0),
                        stop=False,
                    )
                for j in (NS - 2, NS - 1):
                    nc.tensor.matmul(
                        pr[:],
                        ones_bf[:, 0:1],
                        at[j][:],
                        start=False,
                        stop=(j == NS - 1),
                    )
                state["pending"] = evict_chunk(b, qc, po, pr)

        if state["pending"] is not None:
            emit_epilogue(*state["pending"])
            state["pending"] = None


def build_program(fast_mm=True):
    nc = bacc.Bacc("TRN2", target_bir_lowering=False, debug=False)
    x_ap = nc.dram_tensor("x", [BPC, S, D], F32, kind="ExternalInput").ap()
    w_aps = {
        nm: nc.dram_tensor(nm, [D, D], F32, kind="ExternalInput").ap()
        for nm in ("Wq", "Wk", "Wv", "Wo")
    }
    b_aps = {
        nm: nc.dram_tensor(nm, [D], F32, kind="ExternalInput").ap()
        for nm in ("bq", "bk", "bv", "bo")
    }
    y_ap = nc.dram_tensor("y", [BPC, S, D], F32, kind="ExternalOutput").ap()
    with tile.TileContext(nc) as tc:
        _emit(tc, x_ap, w_aps, b_aps, y_ap, fast_mm=fast_mm)
    nc.compile()
    return nc


_program_cache = {}


def _get_program(fast_mm=True):
    if fast_mm not in _program_cache:
        _program_cache[fast_mm] = build_program(fast_mm)
    return _program_cache[fast_mm]


def _make_in_maps(inputs):
    arrs = {
        k: np.ascontiguousarray(np.asarray(v, dtype=np.float32))
        for k, v in inputs.items()
    }
    in_maps = []
    for core in range(N_CORES):
        m = {"x": arrs["x"][BPC * core : BPC * (core + 1)]}
        for nm in ("Wq", "Wk", "Wv", "Wo", "bq", "bk", "bv", "bo"):
            m[nm] = arrs[nm]
        in_maps.append(m)
    return in_maps


def run(inputs, fast_mm=True, trace=False):
    """Returns (y_full, BassKernelResults)."""
    nc = _get_program(fast_mm)
    in_maps = _make_in_maps(inputs)
    last_err = None
    for attempt in range(3):
        try:
            res = run_bass_kernel_spmd(nc, in_maps, list(range(N_CORES)), trace=trace)
            break
        except Exception as e:  # transient NRT device errors: retry
            last_err = e
            import time

            time.sleep(2.0 * (attempt + 1))
    else:
        raise last_err
    y = np.concatenate([r["y"] for r in res.results], axis=0)
    return np.ascontiguousarray(y.astype(np.float32)), res


def kernel(**inputs):
    y, _ = run(inputs, fast_mm=True, trace=False)
    return y
